# revision 5
# baseline (speedup 1.0000x reference)
"""Tensor-parallel causal multi-head attention on one TRN2 chip (8 NeuronCores).

Problem: hidden [B=2, S=2048, H=2048], 16 heads x 128 dim, causal attention,
returns (out, attn_weights).

Sharding (SPMD, no collectives): core c -> batch c//4, head-group c%4
(4 of 16 heads => 512 of 2048 projection columns). Each core:
  phase 1: Q^T, K^T ([head_dim, seq], i.e. transposed) and V ([seq, cols])
           projections from H^T streamed in seq-halves, f32r matmuls.
  phase 2: per head, per 1024-wide q-chunk: scores S^T[k,q] = K^T.T @ Q^T on PE,
           exp on ScalarE (scale=1/sqrt(128) folded in), causal mask via a
           [128,512] 0/1 mask multiply on VectorE (diagonal chunks only),
           row-sums over k (partition axis) via an all-ones stationary matmul
           (result replicated across partitions), 1/sum via Ln then Exp(-x)
           (both live in the natural_log_exp_and_others ACT table set),
           unnormalized AV accumulation on PE, then attn_out = AV * recip and
           P_norm = P * recip on VectorE; P_norm rows DMA'd to attnT[h, k, q].
  phase 3: partial = attn_out_local @ Wo_rows (accumulated over the 4 local
           heads' 128-dim blocks).
Host: out[b] = sum of 4 partials; attn_weights[b,h] = attnT[h%4].T per core.

The strict upper triangle (k > q) of attnT is never written: output DRAM
buffers are pre-zeroed by the runner, and masked-to-zero regions near the
diagonal are written as computed zeros.
"""

import math
import os

import numpy as np

HID = 2048
S = 2048
B = 2
NH = 16
HD = 128
N_CORES = 8
HL = 4            # heads per core
CL = HL * HD      # projection columns per core


def build_bass(s=S, hid=HID, hl=HL, use_f32r=True):
    import concourse.bass as bass  # noqa: F401
    import concourse.tile as tile
    from concourse import bacc, mybir
    from contextlib import ExitStack

    F32 = mybir.dt.float32
    DT = mybir.dt.float32r if use_f32r else F32
    AFT = mybir.ActivationFunctionType
    cl = hl * 128
    KT = hid // 128          # hidden contraction tiles
    ST = s // 128            # seq tiles
    S2 = s // 2              # seq half (phase-1 streaming granularity)
    QC = min(1024, s)        # q-chunk
    NQC = s // QC
    CPQ = max(1, QC // 512)  # 512-chunks per q-chunk
    SCALE = 1.0 / math.sqrt(128.0)

    nc = bacc.Bacc("TRN2", target_bir_lowering=False, debug=False)
    hT = nc.dram_tensor("hT", [hid, s], DT, kind="ExternalInput").ap()
    wq = nc.dram_tensor("wq", [hid, cl], DT, kind="ExternalInput").ap()
    wk = nc.dram_tensor("wk", [hid, cl], DT, kind="ExternalInput").ap()
    wv = nc.dram_tensor("wv", [hid, cl], DT, kind="ExternalInput").ap()
    wo = nc.dram_tensor("wo", [cl, hid], DT, kind="ExternalInput").ap()
    ones = nc.dram_tensor("ones", [128, 128], DT, kind="ExternalInput").ap()
    masks = nc.dram_tensor("masks", [4, 128, 512], DT, kind="ExternalInput").ap()
    attnT = nc.dram_tensor("attnT", [hl, s, s], DT, kind="ExternalOutput").ap()
    partial = nc.dram_tensor("partial", [s, hid], F32, kind="ExternalOutput").ap()

    def R(ap):
        return ap

    with tile.TileContext(nc) as tc, ExitStack() as stk:
        dma = nc.sync.dma_start

        cpool = stk.enter_context(tc.tile_pool(name="const", bufs=1))
        ones_sb = cpool.tile([128, 128], DT, tag="ones", name="ones_sb")
        dma(ones_sb[:], ones[:])
        mask_sb = cpool.tile([128, 4 * 512], DT, tag="mask", name="mask_sb")
        for j in range(4):
            dma(mask_sb[:, j * 512:(j + 1) * 512], masks[j])

        apool = stk.enter_context(tc.tile_pool(name="acts", bufs=1))
        QT = [apool.tile([128, s], DT, tag=f"qt{m}", name=f"qt{m}") for m in range(hl)]
        KTt = [apool.tile([128, s], DT, tag=f"kt{m}", name=f"kt{m}") for m in range(hl)]
        V = [apool.tile([128, cl], DT, tag=f"v{t}", name=f"v{t}") for t in range(ST)]
        AO = [apool.tile([128, s], DT, tag=f"ao{h}", name=f"ao{h}") for h in range(hl)]

        # ---------------- phase 1: projections ----------------
        with tc.tile_pool(name="hbuf", bufs=1) as hpool, \
             tc.tile_pool(name="wbuf", bufs=3) as wpool, \
             tc.tile_pool(name="ppsum", bufs=1, space="PSUM") as ppsum:
            for half in range(2):
                hh = [hpool.tile([128, S2], DT, tag=f"h{k}", name=f"h{k}") for k in range(KT)]
                for k in range(KT):
                    dma(hh[k][:], hT[128 * k:128 * (k + 1), half * S2:(half + 1) * S2])
                # Q^T and K^T: stationary = W column tile, moving = H^T
                for wdram, outt in ((wq, QT), (wk, KTt)):
                    ps = [ppsum.tile([128, S2], F32, tag=f"pp{m}", name=f"pp{m}") for m in range(hl)]
                    for k in range(KT):
                        wt = wpool.tile([128, cl], DT, tag="w", name="wt")
                        dma(wt[:], wdram[128 * k:128 * (k + 1), :])
                        for m in range(hl):
                            for n0 in range(0, S2, 512):
                                nn = min(512, S2 - n0)
                                nc.tensor.matmul(
                                    ps[m][:, n0:n0 + nn],
                                    R(wt[:, 128 * m:128 * (m + 1)]),
                                    R(hh[k][:, n0:n0 + nn]),
                                    start=(k == 0), stop=(k == KT - 1))
                    for m in range(hl):
                        nc.scalar.copy(outt[m][:, half * S2:(half + 1) * S2], ps[m][:])
                # V: stationary = H^T seq tile, moving = Wv. PSUM reuses the
                # pp{j} tags: seq tile t lives in tile t%hl, slot t//hl.
                nvs = S2 // 128
                vtiles = [ppsum.tile([128, S2], F32, tag=f"pp{j}", name=f"vp{j}")
                          for j in range(hl)]
                vps = [vtiles[t % hl][:, (t // hl) * cl:(t // hl + 1) * cl]
                       for t in range(nvs)]
                for k in range(KT):
                    wt = wpool.tile([128, cl], DT, tag="w", name="wt")
                    dma(wt[:], wv[128 * k:128 * (k + 1), :])
                    for t in range(nvs):
                        nc.tensor.matmul(
                            vps[t][:],
                            R(hh[k][:, 128 * t:128 * (t + 1)]),
                            R(wt[:]),
                            start=(k == 0), stop=(k == KT - 1))
                for t in range(nvs):
                    nc.scalar.copy(V[half * nvs + t][:], vps[t][:])

        # ---------------- phase 2: attention ----------------
        with tc.tile_pool(name="pt", bufs=1) as ptpool, \
             tc.tile_pool(name="misc", bufs=1) as mpool, \
             tc.tile_pool(name="scps", bufs=2, space="PSUM") as scpool, \
             tc.tile_pool(name="abps", bufs=1, space="PSUM") as abpsum:
            for h in range(hl):
                for qc in range(NQC):
                    qlo = qc * QC
                    ch0 = qc * CPQ
                    ch1 = (qc + 1) * CPQ - 1
                    i_hi = min(ST, 4 * ch1 + 4)
                    sum_ps = abpsum.tile([128, QC], F32, tag="sum", name="sum_ps")
                    av_ps = abpsum.tile([128, QC], F32, tag="av", name="av_ps")
                    pts = {}
                    for i in range(i_hi):
                        c0 = max(i // 4, ch0)
                        ext = (ch1 + 1 - c0) * 512
                        qoff = c0 * 512
                        sc = scpool.tile([128, ext], F32, tag="sc", name="sc")
                        for cc in range(c0, ch1 + 1):
                            o = (cc - c0) * 512
                            nc.tensor.matmul(
                                sc[:, o:o + 512],
                                R(KTt[h][:, 128 * i:128 * (i + 1)]),
                                R(QT[h][:, cc * 512:(cc + 1) * 512]),
                                start=True, stop=True)
                        pt = ptpool.tile([128, ext], DT, tag=f"pt{i}", name=f"pt{i}")
                        nc.scalar.activation(pt[:], sc[:], AFT.Exp, scale=SCALE)
                        if i // 4 >= ch0:
                            j = i % 4
                            nc.vector.tensor_mul(
                                pt[:, 0:512], pt[:, 0:512],
                                mask_sb[:, j * 512:(j + 1) * 512])
                        for cc in range(c0, ch1 + 1):
                            o = (cc - c0) * 512
                            qs = cc * 512 - qlo
                            stop_i = min(4 * cc + 3, i_hi - 1)
                            nc.tensor.matmul(
                                sum_ps[:, qs:qs + 512], R(ones_sb[:]),
                                R(pt[:, o:o + 512]),
                                start=(i == 0), stop=(i == stop_i))
                            nc.tensor.matmul(
                                av_ps[:, qs:qs + 512],
                                R(V[i][:, 128 * h:128 * (h + 1)]),
                                R(pt[:, o:o + 512]),
                                start=(i == 0), stop=(i == stop_i))
                        pts[i] = (pt, ext, qoff)
                    lns = mpool.tile([128, QC], F32, tag="lns", name="lns")
                    nc.scalar.activation(lns[:], sum_ps[:], AFT.Ln)
                    rec = mpool.tile([128, QC], DT, tag="rec", name="rec")
                    nc.scalar.activation(rec[:], lns[:], AFT.Exp, scale=-1.0)
                    nc.vector.tensor_mul(AO[h][:, qlo:qlo + QC], av_ps[:], rec[:])
                    for i in range(i_hi):
                        pt, ext, qoff = pts[i]
                        nc.vector.tensor_mul(
                            pt[:], pt[:], rec[:, qoff - qlo:qoff - qlo + ext])
                        dma(attnT[h, 128 * i:128 * (i + 1), qoff:qoff + ext], pt[:])

        # ---------------- phase 3: output projection ----------------
        with tc.tile_pool(name="wo", bufs=1) as wopool, \
             tc.tile_pool(name="oout", bufs=4) as opool, \
             tc.tile_pool(name="ops", bufs=4, space="PSUM") as opsum:
            wos = [wopool.tile([128, hid], DT, tag=f"wo{m}", name=f"wos{m}") for m in range(hl)]
            for m in range(hl):
                dma(wos[m][:], wo[128 * m:128 * (m + 1), :])
            for qt in range(ST):
                for e0 in range(0, hid, 512):
                    ee = min(512, hid - e0)
                    ps = opsum.tile([128, 512], F32, tag="op", name="ops")
                    for m in range(hl):
                        nc.tensor.matmul(
                            ps[:, 0:ee],
                            R(AO[m][:, 128 * qt:128 * (qt + 1)]),
                            R(wos[m][:, e0:e0 + ee]),
                            start=(m == 0), stop=(m == hl - 1))
                    ot = opool.tile([128, 512], F32, tag="ot", name="ot")
                    nc.any.tensor_copy(ot[:, 0:ee], ps[:, 0:ee])
                    dma(partial[128 * qt:128 * (qt + 1), e0:e0 + ee], ot[:, 0:ee])

    nc.compile()
    return nc


def _host_inputs(hidden, Wq, Wk, Wv, Wo):
    hidden = np.asarray(hidden, dtype=np.float32)
    Wq = np.asarray(Wq, dtype=np.float32)
    Wk = np.asarray(Wk, dtype=np.float32)
    Wv = np.asarray(Wv, dtype=np.float32)
    Wo = np.asarray(Wo, dtype=np.float32)
    hTs = [np.ascontiguousarray(hidden[b].T) for b in range(hidden.shape[0])]
    ones = np.ones((128, 128), dtype=np.float32)
    kk = np.arange(128)[:, None]
    qq = np.arange(512)[None, :]
    masks = np.stack(
        [(qq >= 128 * j + kk) for j in range(4)]).astype(np.float32)
    in_maps = []
    for c in range(N_CORES):
        b, g = divmod(c, 4)
        sl = slice(CL * g, CL * (g + 1))
        in_maps.append({
            "hT": hTs[b],
            "wq": np.ascontiguousarray(Wq[:, sl]),
            "wk": np.ascontiguousarray(Wk[:, sl]),
            "wv": np.ascontiguousarray(Wv[:, sl]),
            "wo": np.ascontiguousarray(Wo[sl, :]),
            "ones": ones,
            "masks": masks,
        })
    return in_maps


_CACHE = {}


def kernel(hidden_states, Wq, Wk, Wv, Wo):
    from concourse.bass_utils import run_bass_kernel_spmd

    if "nc" not in _CACHE:
        _CACHE["nc"] = build_bass()
    nc = _CACHE["nc"]

    in_maps = _host_inputs(hidden_states, Wq, Wk, Wv, Wo)
    trace = bool(int(os.environ.get("BASS_KERNEL_TRACE", "0")))
    tmpdir = os.environ.get("BASS_KERNEL_TRACE_DIR") or None
    res = run_bass_kernel_spmd(
        nc, in_maps, core_ids=list(range(N_CORES)), trace=trace, tmpdir=tmpdir)
    _CACHE["last_results"] = res

    out = np.zeros((B, S, HID), dtype=np.float32)
    attn = np.empty((B, NH, S, S), dtype=np.float32)
    for c in range(N_CORES):
        b, g = divmod(c, 4)
        r = res.results[c]
        out[b] += r["partial"]
        at = r["attnT"]
        for hli in range(HL):
            attn[b, HL * g + hli] = at[hli].T
    return out, attn


# revision 10
# speedup vs baseline: 1.0958x; 1.0958x over previous
"""Tensor-parallel causal multi-head attention on one TRN2 chip (8 NeuronCores).

Problem: hidden [B=2, S=2048, H=2048], 16 heads x 128 dim, causal attention,
returns (out, attn_weights).

Sharding (SPMD, no collectives): core c -> batch c//4, head-group c%4
(4 of 16 heads => 512 of 2048 projection columns). Each core:
  phase 1: Q^T, K^T ([head_dim, seq], i.e. transposed) and V ([seq, cols])
           projections from H^T streamed in seq-halves, f32r matmuls.
  phase 2: per head, per 1024-wide q-chunk: scores S^T[k,q] = K^T.T @ Q^T on PE,
           exp on ScalarE (scale=1/sqrt(128) folded in), causal mask via a
           [128,512] 0/1 mask multiply on VectorE (diagonal chunks only),
           row-sums over k (partition axis) via an all-ones stationary matmul
           (result replicated across partitions), 1/sum via Ln then Exp(-x)
           (both live in the natural_log_exp_and_others ACT table set),
           unnormalized AV accumulation on PE, then attn_out = AV * recip and
           P_norm = P * recip on VectorE; P_norm rows DMA'd to attnT[h, k, q].
  phase 3: partial = attn_out_local @ Wo_rows (accumulated over the 4 local
           heads' 128-dim blocks).
Host: out[b] = sum of 4 partials; attn_weights[b,h] = attnT[h%4].T per core.

The strict upper triangle (k > q) of attnT is never written: output DRAM
buffers are pre-zeroed by the runner, and masked-to-zero regions near the
diagonal are written as computed zeros.
"""

import math
import os

import numpy as np

HID = 2048
S = 2048
B = 2
NH = 16
HD = 128
N_CORES = 8
HL = 4            # heads per core
CL = HL * HD      # projection columns per core


def build_bass(s=S, hid=HID, hl=HL, use_f32r=True):
    import concourse.bass as bass  # noqa: F401
    import concourse.tile as tile
    from concourse import bacc, mybir
    from contextlib import ExitStack

    F32 = mybir.dt.float32
    DT = mybir.dt.float32r if use_f32r else F32
    AFT = mybir.ActivationFunctionType
    cl = hl * 128
    KT = hid // 128          # hidden contraction tiles
    ST = s // 128            # seq tiles
    S2 = s // 2              # seq half (phase-1 streaming granularity)
    QC = min(1024, s)        # q-chunk
    NQC = s // QC
    CPQ = max(1, QC // 512)  # 512-chunks per q-chunk
    SCALE = 1.0 / math.sqrt(128.0)

    nc = bacc.Bacc("TRN2", target_bir_lowering=False, debug=False)
    hT = nc.dram_tensor("hT", [hid, s], DT, kind="ExternalInput").ap()
    wq = nc.dram_tensor("wq", [hid, cl], DT, kind="ExternalInput").ap()
    wk = nc.dram_tensor("wk", [hid, cl], DT, kind="ExternalInput").ap()
    wv = nc.dram_tensor("wv", [hid, cl], DT, kind="ExternalInput").ap()
    wo = nc.dram_tensor("wo", [cl, hid], DT, kind="ExternalInput").ap()
    ones = nc.dram_tensor("ones", [128, 128], DT, kind="ExternalInput").ap()
    masks = nc.dram_tensor("masks", [4, 128, 512], DT, kind="ExternalInput").ap()
    attnT = nc.dram_tensor("attnT", [hl, s, s], DT, kind="ExternalOutput").ap()
    partial = nc.dram_tensor("partial", [s, hid], F32, kind="ExternalOutput").ap()

    def R(ap):
        return ap

    with tile.TileContext(nc) as tc, ExitStack() as stk:
        # separate HWDGE queues: loads on the Activation queue, stores on
        # the SP queue, so input prefetch never sits behind a store backlog.
        dma = nc.scalar.dma_start
        dma_st = nc.sync.dma_start

        cpool = stk.enter_context(tc.tile_pool(name="const", bufs=1))
        ones_sb = cpool.tile([128, 128], DT, tag="ones", name="ones_sb")
        dma(ones_sb[:], ones[:])
        mask_sb = cpool.tile([128, 4 * 512], DT, tag="mask", name="mask_sb")
        for j in range(4):
            dma(mask_sb[:, j * 512:(j + 1) * 512], masks[j])

        apool = stk.enter_context(tc.tile_pool(name="acts", bufs=1))
        QT = [apool.tile([128, s], DT, tag=f"qt{m}", name=f"qt{m}") for m in range(hl)]
        KTt = [apool.tile([128, s], DT, tag=f"kt{m}", name=f"kt{m}") for m in range(hl)]
        V = [apool.tile([128, cl], DT, tag=f"v{t}", name=f"v{t}") for t in range(ST)]
        AO = [apool.tile([128, s], DT, tag=f"ao{h}", name=f"ao{h}") for h in range(hl)]

        # ---------------- phase 1: projections ----------------
        with tc.tile_pool(name="hbuf", bufs=1) as hpool, \
             tc.tile_pool(name="wbuf", bufs=3) as wpool, \
             tc.tile_pool(name="ppsum", bufs=1, space="PSUM") as ppsum:
            for half in range(2):
                hh = [hpool.tile([128, S2], DT, tag=f"h{k}", name=f"h{k}") for k in range(KT)]
                for k in range(KT):
                    dma(hh[k][:], hT[128 * k:128 * (k + 1), half * S2:(half + 1) * S2])
                # Q^T and K^T: stationary = W column tile, moving = H^T
                for wdram, outt in ((wq, QT), (wk, KTt)):
                    ps = [ppsum.tile([128, S2], F32, tag=f"pp{m}", name=f"pp{m}") for m in range(hl)]
                    for k in range(KT):
                        wt = wpool.tile([128, cl], DT, tag="w", name="wt")
                        dma(wt[:], wdram[128 * k:128 * (k + 1), :])
                        for m in range(hl):
                            for n0 in range(0, S2, 512):
                                nn = min(512, S2 - n0)
                                nc.tensor.matmul(
                                    ps[m][:, n0:n0 + nn],
                                    R(wt[:, 128 * m:128 * (m + 1)]),
                                    R(hh[k][:, n0:n0 + nn]),
                                    start=(k == 0), stop=(k == KT - 1))
                    for m in range(hl):
                        if m % 2 == 0:
                            nc.scalar.copy(
                                outt[m][:, half * S2:(half + 1) * S2], ps[m][:])
                        else:
                            nc.vector.tensor_copy(
                                outt[m][:, half * S2:(half + 1) * S2], ps[m][:])
                # V: stationary = H^T seq tile, moving = Wv. PSUM reuses the
                # pp{j} tags: seq tile t lives in tile t%hl, slot t//hl.
                nvs = S2 // 128
                vtiles = [ppsum.tile([128, S2], F32, tag=f"pp{j}", name=f"vp{j}")
                          for j in range(hl)]
                vps = [vtiles[t % hl][:, (t // hl) * cl:(t // hl + 1) * cl]
                       for t in range(nvs)]
                for k in range(KT):
                    wt = wpool.tile([128, cl], DT, tag="w", name="wt")
                    dma(wt[:], wv[128 * k:128 * (k + 1), :])
                    for t in range(nvs):
                        nc.tensor.matmul(
                            vps[t][:],
                            R(hh[k][:, 128 * t:128 * (t + 1)]),
                            R(wt[:]),
                            start=(k == 0), stop=(k == KT - 1))
                for t in range(nvs):
                    if t % 2 == 0:
                        nc.scalar.copy(V[half * nvs + t][:], vps[t][:])
                    else:
                        nc.vector.tensor_copy(V[half * nvs + t][:], vps[t][:])

        # ---------------- phase 2: attention ----------------
        with tc.tile_pool(name="pt", bufs=1) as ptpool, \
             tc.tile_pool(name="misc", bufs=1) as mpool, \
             tc.tile_pool(name="scps", bufs=2, space="PSUM") as scpool, \
             tc.tile_pool(name="abps", bufs=1, space="PSUM") as abpsum:
            for h in range(hl):
                for qc in range(NQC):
                    ch0 = qc * CPQ
                    ch1 = (qc + 1) * CPQ - 1
                    i_hi = min(ST, 4 * ch1 + 4)
                    sum_t = {cc: abpsum.tile([128, 512], F32, tag="sum",
                                             bufs=2, name=f"sum{cc}")
                             for cc in range(ch0, ch1 + 1)}
                    av_t = {cc: abpsum.tile([128, 512], F32, tag="av",
                                            bufs=2, name=f"av{cc}")
                            for cc in range(ch0, ch1 + 1)}
                    pts = {}
                    for i in range(i_hi):
                        c0 = max(i // 4, ch0)
                        ext = (ch1 + 1 - c0) * 512
                        qoff = c0 * 512
                        sc = scpool.tile([128, ext], F32, tag="sc", name="sc")
                        for cc in range(c0, ch1 + 1):
                            o = (cc - c0) * 512
                            nc.tensor.matmul(
                                sc[:, o:o + 512],
                                R(KTt[h][:, 128 * i:128 * (i + 1)]),
                                R(QT[h][:, cc * 512:(cc + 1) * 512]),
                                start=True, stop=True)
                        pt = ptpool.tile([128, ext], DT, tag=f"pt{i}", name=f"pt{i}")
                        nc.scalar.activation(pt[:], sc[:], AFT.Exp, scale=SCALE)
                        if i // 4 >= ch0:
                            j = i % 4
                            nc.vector.tensor_mul(
                                pt[:, 0:512], pt[:, 0:512],
                                mask_sb[:, j * 512:(j + 1) * 512])
                        for cc in range(c0, ch1 + 1):
                            o = (cc - c0) * 512
                            stop_i = min(4 * cc + 3, i_hi - 1)
                            nc.tensor.matmul(
                                sum_t[cc][:], R(ones_sb[:]),
                                R(pt[:, o:o + 512]),
                                start=(i == 0), stop=(i == stop_i))
                            nc.tensor.matmul(
                                av_t[cc][:],
                                R(V[i][:, 128 * h:128 * (h + 1)]),
                                R(pt[:, o:o + 512]),
                                start=(i == 0), stop=(i == stop_i))
                        pts[i] = (pt, ext, qoff)
                    recs = {}
                    for cc in range(ch0, ch1 + 1):
                        lns = mpool.tile([128, 512], F32, tag="lns",
                                         bufs=2, name=f"lns{cc}")
                        nc.scalar.activation(lns[:], sum_t[cc][:], AFT.Ln)
                        rec = mpool.tile([128, 512], F32, tag="rec",
                                         bufs=2, name=f"rec{cc}")
                        nc.scalar.activation(rec[:], lns[:], AFT.Exp, scale=-1.0)
                        nc.vector.tensor_mul(
                            AO[h][:, cc * 512:(cc + 1) * 512], av_t[cc][:], rec[:])
                        recs[cc] = rec
                    for i in range(i_hi):
                        pt, ext, qoff = pts[i]
                        c0 = qoff // 512
                        for cc in range(c0, ch1 + 1):
                            o = (cc - c0) * 512
                            nc.vector.tensor_mul(
                                pt[:, o:o + 512], pt[:, o:o + 512], recs[cc][:])
                        dma_st(attnT[h, 128 * i:128 * (i + 1), qoff:qoff + ext],
                               pt[:])

        # ---------------- phase 3: output projection ----------------
        with tc.tile_pool(name="wo", bufs=1) as wopool, \
             tc.tile_pool(name="oout", bufs=4) as opool, \
             tc.tile_pool(name="ops", bufs=4, space="PSUM") as opsum:
            wos = [wopool.tile([128, hid], DT, tag=f"wo{m}", name=f"wos{m}") for m in range(hl)]
            for m in range(hl):
                dma(wos[m][:], wo[128 * m:128 * (m + 1), :])
            for qt in range(ST):
                for e0 in range(0, hid, 512):
                    ee = min(512, hid - e0)
                    ps = opsum.tile([128, 512], F32, tag="op", name="ops")
                    for m in range(hl):
                        nc.tensor.matmul(
                            ps[:, 0:ee],
                            R(AO[m][:, 128 * qt:128 * (qt + 1)]),
                            R(wos[m][:, e0:e0 + ee]),
                            start=(m == 0), stop=(m == hl - 1))
                    ot = opool.tile([128, 512], F32, tag="ot", name="ot")
                    nc.vector.tensor_copy(ot[:, 0:ee], ps[:, 0:ee])
                    dma_st(partial[128 * qt:128 * (qt + 1), e0:e0 + ee], ot[:, 0:ee])

    nc.compile()
    return nc


def _host_inputs(hidden, Wq, Wk, Wv, Wo):
    hidden = np.asarray(hidden, dtype=np.float32)
    Wq = np.asarray(Wq, dtype=np.float32)
    Wk = np.asarray(Wk, dtype=np.float32)
    Wv = np.asarray(Wv, dtype=np.float32)
    Wo = np.asarray(Wo, dtype=np.float32)
    hTs = [np.ascontiguousarray(hidden[b].T) for b in range(hidden.shape[0])]
    ones = np.ones((128, 128), dtype=np.float32)
    kk = np.arange(128)[:, None]
    qq = np.arange(512)[None, :]
    masks = np.stack(
        [(qq >= 128 * j + kk) for j in range(4)]).astype(np.float32)
    in_maps = []
    for c in range(N_CORES):
        b, g = divmod(c, 4)
        sl = slice(CL * g, CL * (g + 1))
        in_maps.append({
            "hT": hTs[b],
            "wq": np.ascontiguousarray(Wq[:, sl]),
            "wk": np.ascontiguousarray(Wk[:, sl]),
            "wv": np.ascontiguousarray(Wv[:, sl]),
            "wo": np.ascontiguousarray(Wo[sl, :]),
            "ones": ones,
            "masks": masks,
        })
    return in_maps


_CACHE = {}


def kernel(hidden_states, Wq, Wk, Wv, Wo):
    from concourse.bass_utils import run_bass_kernel_spmd

    if "nc" not in _CACHE:
        _CACHE["nc"] = build_bass()
    nc = _CACHE["nc"]

    in_maps = _host_inputs(hidden_states, Wq, Wk, Wv, Wo)
    trace = bool(int(os.environ.get("BASS_KERNEL_TRACE", "0")))
    tmpdir = os.environ.get("BASS_KERNEL_TRACE_DIR") or None
    res = run_bass_kernel_spmd(
        nc, in_maps, core_ids=list(range(N_CORES)), trace=trace, tmpdir=tmpdir)
    _CACHE["last_results"] = res

    out = np.zeros((B, S, HID), dtype=np.float32)
    attn = np.empty((B, NH, S, S), dtype=np.float32)
    for c in range(N_CORES):
        b, g = divmod(c, 4)
        r = res.results[c]
        out[b] += r["partial"]
        at = r["attnT"]
        for hli in range(HL):
            attn[b, HL * g + hli] = at[hli].T
    return out, attn


# revision 11
# speedup vs baseline: 1.1634x; 1.0617x over previous
"""Tensor-parallel causal multi-head attention on one TRN2 chip (8 NeuronCores).

Problem: hidden [B=2, S=2048, H=2048], 16 heads x 128 dim, causal attention,
returns (out, attn_weights).

Sharding (SPMD, no collectives): core c -> batch c//4, head-group c%4
(4 of 16 heads => 512 of 2048 projection columns). Each core:
  phase 1: Q^T, K^T ([head_dim, seq], i.e. transposed) and V ([seq, cols])
           projections from H^T streamed in seq-halves, f32r matmuls.
  phase 2: per head, per 1024-wide q-chunk: scores S^T[k,q] = K^T.T @ Q^T on PE,
           exp on ScalarE (scale=1/sqrt(128) folded in), causal mask via a
           [128,512] 0/1 mask multiply on VectorE (diagonal chunks only),
           row-sums over k (partition axis) via an all-ones stationary matmul
           (result replicated across partitions), 1/sum via Ln then Exp(-x)
           (both live in the natural_log_exp_and_others ACT table set),
           unnormalized AV accumulation on PE, then attn_out = AV * recip and
           P_norm = P * recip on VectorE; P_norm rows DMA'd to attnT[h, k, q].
  phase 3: partial = attn_out_local @ Wo_rows (accumulated over the 4 local
           heads' 128-dim blocks).
Host: out[b] = sum of 4 partials; attn_weights[b,h] = attnT[h%4].T per core.

The strict upper triangle (k > q) of attnT is never written: output DRAM
buffers are pre-zeroed by the runner, and masked-to-zero regions near the
diagonal are written as computed zeros.
"""

import math
import os

import numpy as np

HID = 2048
S = 2048
B = 2
NH = 16
HD = 128
N_CORES = 8
HL = 4            # heads per core
CL = HL * HD      # projection columns per core


def build_bass(s=S, hid=HID, hl=HL, use_f32r=True):
    import concourse.bass as bass  # noqa: F401
    import concourse.tile as tile
    from concourse import bacc, mybir
    from contextlib import ExitStack

    F32 = mybir.dt.float32
    DT = mybir.dt.float32r if use_f32r else F32
    AFT = mybir.ActivationFunctionType
    cl = hl * 128
    KT = hid // 128          # hidden contraction tiles
    ST = s // 128            # seq tiles
    S2 = s // 2              # seq half (phase-1 streaming granularity)
    QC = min(1024, s)        # q-chunk
    NQC = s // QC
    CPQ = max(1, QC // 512)  # 512-chunks per q-chunk
    SCALE = 1.0 / math.sqrt(128.0)

    nc = bacc.Bacc("TRN2", target_bir_lowering=False, debug=False)
    hT = nc.dram_tensor("hT", [hid, s], DT, kind="ExternalInput").ap()
    wq = nc.dram_tensor("wq", [hid, cl], DT, kind="ExternalInput").ap()
    wk = nc.dram_tensor("wk", [hid, cl], DT, kind="ExternalInput").ap()
    wv = nc.dram_tensor("wv", [hid, cl], DT, kind="ExternalInput").ap()
    wo = nc.dram_tensor("wo", [cl, hid], DT, kind="ExternalInput").ap()
    ones = nc.dram_tensor("ones", [128, 128], DT, kind="ExternalInput").ap()
    masks = nc.dram_tensor("masks", [4, 128, 512], DT, kind="ExternalInput").ap()
    attnT = nc.dram_tensor("attnT", [hl, s, s], DT, kind="ExternalOutput").ap()
    partial = nc.dram_tensor("partial", [s, hid], F32, kind="ExternalOutput").ap()

    def R(ap):
        return ap

    with tile.TileContext(nc) as tc, ExitStack() as stk:
        # separate HWDGE queues: loads on the Activation queue, stores on
        # the SP queue, so input prefetch never sits behind a store backlog.
        dma = nc.scalar.dma_start
        dma_st = nc.sync.dma_start

        cpool = stk.enter_context(tc.tile_pool(name="const", bufs=1))
        ones_sb = cpool.tile([128, 128], DT, tag="ones", name="ones_sb")
        dma(ones_sb[:], ones[:])
        mask_sb = cpool.tile([128, 4 * 512], DT, tag="mask", name="mask_sb")
        for j in range(4):
            dma(mask_sb[:, j * 512:(j + 1) * 512], masks[j])

        apool = stk.enter_context(tc.tile_pool(name="acts", bufs=1))
        QT = [apool.tile([128, s], DT, tag=f"qt{m}", name=f"qt{m}") for m in range(hl)]
        KTt = [apool.tile([128, s], DT, tag=f"kt{m}", name=f"kt{m}") for m in range(hl)]
        V = [apool.tile([128, cl], DT, tag=f"v{t}", name=f"v{t}") for t in range(ST)]
        AO = [apool.tile([128, s], DT, tag=f"ao{h}", name=f"ao{h}") for h in range(hl)]

        # ---------------- phase 1: projections ----------------
        with tc.tile_pool(name="hbuf", bufs=1) as hpool, \
             tc.tile_pool(name="wbuf", bufs=3) as wpool, \
             tc.tile_pool(name="ppsum", bufs=1, space="PSUM") as ppsum:
            for half in range(2):
                hh = [hpool.tile([128, S2], DT, tag=f"h{k}", name=f"h{k}") for k in range(KT)]
                for k in range(KT):
                    dma_st(hh[k][:],
                           hT[128 * k:128 * (k + 1), half * S2:(half + 1) * S2])
                # Q^T and K^T: stationary = W column tile, moving = H^T
                for wdram, outt in ((wq, QT), (wk, KTt)):
                    ps = [ppsum.tile([128, S2], F32, tag=f"pp{m}", name=f"pp{m}") for m in range(hl)]
                    for k in range(KT):
                        wt = wpool.tile([128, cl], DT, tag="w", name="wt")
                        dma(wt[:], wdram[128 * k:128 * (k + 1), :])
                        for m in range(hl):
                            for n0 in range(0, S2, 512):
                                nn = min(512, S2 - n0)
                                nc.tensor.matmul(
                                    ps[m][:, n0:n0 + nn],
                                    R(wt[:, 128 * m:128 * (m + 1)]),
                                    R(hh[k][:, n0:n0 + nn]),
                                    start=(k == 0), stop=(k == KT - 1))
                    for m in range(hl):
                        if m % 2 == 0:
                            nc.scalar.copy(
                                outt[m][:, half * S2:(half + 1) * S2], ps[m][:])
                        else:
                            nc.vector.tensor_copy(
                                outt[m][:, half * S2:(half + 1) * S2], ps[m][:])
                # V: stationary = H^T seq tile, moving = Wv. PSUM reuses the
                # pp{j} tags: seq tile t lives in tile t%hl, slot t//hl.
                nvs = S2 // 128
                vtiles = [ppsum.tile([128, S2], F32, tag=f"pp{j}", name=f"vp{j}")
                          for j in range(hl)]
                vps = [vtiles[t % hl][:, (t // hl) * cl:(t // hl + 1) * cl]
                       for t in range(nvs)]
                for k in range(KT):
                    wt = wpool.tile([128, cl], DT, tag="w", name="wt")
                    dma(wt[:], wv[128 * k:128 * (k + 1), :])
                    for t in range(nvs):
                        nc.tensor.matmul(
                            vps[t][:],
                            R(hh[k][:, 128 * t:128 * (t + 1)]),
                            R(wt[:]),
                            start=(k == 0), stop=(k == KT - 1))
                for t in range(nvs):
                    if t % 2 == 0:
                        nc.scalar.copy(V[half * nvs + t][:], vps[t][:])
                    else:
                        nc.vector.tensor_copy(V[half * nvs + t][:], vps[t][:])

        # ---------------- phase 2: attention ----------------
        with tc.tile_pool(name="pt", bufs=1) as ptpool, \
             tc.tile_pool(name="misc", bufs=1) as mpool, \
             tc.tile_pool(name="scps", bufs=2, space="PSUM") as scpool, \
             tc.tile_pool(name="abps", bufs=1, space="PSUM") as abpsum:
            for h in range(hl):
                for qc in range(NQC):
                    ch0 = qc * CPQ
                    ch1 = (qc + 1) * CPQ - 1
                    i_hi = min(ST, 4 * ch1 + 4)
                    sum_t = {cc: abpsum.tile([128, 512], F32, tag="sum",
                                             bufs=2, name=f"sum{cc}")
                             for cc in range(ch0, ch1 + 1)}
                    av_t = {cc: abpsum.tile([128, 512], F32, tag="av",
                                            bufs=2, name=f"av{cc}")
                            for cc in range(ch0, ch1 + 1)}
                    pts = {}
                    for i in range(i_hi):
                        c0 = max(i // 4, ch0)
                        ext = (ch1 + 1 - c0) * 512
                        qoff = c0 * 512
                        sc = scpool.tile([128, ext], F32, tag="sc", name="sc")
                        for cc in range(c0, ch1 + 1):
                            o = (cc - c0) * 512
                            nc.tensor.matmul(
                                sc[:, o:o + 512],
                                R(KTt[h][:, 128 * i:128 * (i + 1)]),
                                R(QT[h][:, cc * 512:(cc + 1) * 512]),
                                start=True, stop=True)
                        pt = ptpool.tile([128, ext], DT, tag=f"pt{i}", name=f"pt{i}")
                        nc.scalar.activation(pt[:], sc[:], AFT.Exp, scale=SCALE)
                        if i // 4 >= ch0:
                            j = i % 4
                            nc.vector.tensor_mul(
                                pt[:, 0:512], pt[:, 0:512],
                                mask_sb[:, j * 512:(j + 1) * 512])
                        for cc in range(c0, ch1 + 1):
                            o = (cc - c0) * 512
                            stop_i = min(4 * cc + 3, i_hi - 1)
                            nc.tensor.matmul(
                                sum_t[cc][:], R(ones_sb[:]),
                                R(pt[:, o:o + 512]),
                                start=(i == 0), stop=(i == stop_i))
                            nc.tensor.matmul(
                                av_t[cc][:],
                                R(V[i][:, 128 * h:128 * (h + 1)]),
                                R(pt[:, o:o + 512]),
                                start=(i == 0), stop=(i == stop_i))
                        pts[i] = (pt, ext, qoff)
                    recs = {}
                    for cc in range(ch0, ch1 + 1):
                        rec = mpool.tile([128, 512], F32, tag="rec",
                                         bufs=2, name=f"rec{cc}")
                        nc.vector.reciprocal_approx_fast(rec[:], sum_t[cc][:])
                        nc.vector.tensor_mul(
                            AO[h][:, cc * 512:(cc + 1) * 512], av_t[cc][:], rec[:])
                        recs[cc] = rec
                    for i in range(i_hi):
                        pt, ext, qoff = pts[i]
                        c0 = qoff // 512
                        for cc in range(c0, ch1 + 1):
                            o = (cc - c0) * 512
                            nc.vector.tensor_mul(
                                pt[:, o:o + 512], pt[:, o:o + 512], recs[cc][:])
                        dma_st(attnT[h, 128 * i:128 * (i + 1), qoff:qoff + ext],
                               pt[:])

        # ---------------- phase 3: output projection ----------------
        with tc.tile_pool(name="wo", bufs=1) as wopool, \
             tc.tile_pool(name="oout", bufs=4) as opool, \
             tc.tile_pool(name="ops", bufs=4, space="PSUM") as opsum:
            wos = [wopool.tile([128, hid], DT, tag=f"wo{m}", name=f"wos{m}") for m in range(hl)]
            for m in range(hl):
                dma_st(wos[m][:], wo[128 * m:128 * (m + 1), :])
            for qt in range(ST):
                for e0 in range(0, hid, 512):
                    ee = min(512, hid - e0)
                    ps = opsum.tile([128, 512], F32, tag="op", name="ops")
                    for m in range(hl):
                        nc.tensor.matmul(
                            ps[:, 0:ee],
                            R(AO[m][:, 128 * qt:128 * (qt + 1)]),
                            R(wos[m][:, e0:e0 + ee]),
                            start=(m == 0), stop=(m == hl - 1))
                    ot = opool.tile([128, 512], F32, tag="ot", name="ot")
                    if (qt + e0 // 512) % 2 == 0:
                        nc.vector.tensor_copy(ot[:, 0:ee], ps[:, 0:ee])
                    else:
                        nc.scalar.copy(ot[:, 0:ee], ps[:, 0:ee])
                    dma_st(partial[128 * qt:128 * (qt + 1), e0:e0 + ee], ot[:, 0:ee])

    nc.compile()
    return nc


def _host_inputs(hidden, Wq, Wk, Wv, Wo):
    hidden = np.asarray(hidden, dtype=np.float32)
    Wq = np.asarray(Wq, dtype=np.float32)
    Wk = np.asarray(Wk, dtype=np.float32)
    Wv = np.asarray(Wv, dtype=np.float32)
    Wo = np.asarray(Wo, dtype=np.float32)
    hTs = [np.ascontiguousarray(hidden[b].T) for b in range(hidden.shape[0])]
    ones = np.ones((128, 128), dtype=np.float32)
    kk = np.arange(128)[:, None]
    qq = np.arange(512)[None, :]
    masks = np.stack(
        [(qq >= 128 * j + kk) for j in range(4)]).astype(np.float32)
    in_maps = []
    for c in range(N_CORES):
        b, g = divmod(c, 4)
        sl = slice(CL * g, CL * (g + 1))
        in_maps.append({
            "hT": hTs[b],
            "wq": np.ascontiguousarray(Wq[:, sl]),
            "wk": np.ascontiguousarray(Wk[:, sl]),
            "wv": np.ascontiguousarray(Wv[:, sl]),
            "wo": np.ascontiguousarray(Wo[sl, :]),
            "ones": ones,
            "masks": masks,
        })
    return in_maps


_CACHE = {}


def kernel(hidden_states, Wq, Wk, Wv, Wo):
    from concourse.bass_utils import run_bass_kernel_spmd

    if "nc" not in _CACHE:
        _CACHE["nc"] = build_bass()
    nc = _CACHE["nc"]

    in_maps = _host_inputs(hidden_states, Wq, Wk, Wv, Wo)
    trace = bool(int(os.environ.get("BASS_KERNEL_TRACE", "0")))
    tmpdir = os.environ.get("BASS_KERNEL_TRACE_DIR") or None
    res = run_bass_kernel_spmd(
        nc, in_maps, core_ids=list(range(N_CORES)), trace=trace, tmpdir=tmpdir)
    _CACHE["last_results"] = res

    out = np.zeros((B, S, HID), dtype=np.float32)
    attn = np.empty((B, NH, S, S), dtype=np.float32)
    for c in range(N_CORES):
        b, g = divmod(c, 4)
        r = res.results[c]
        out[b] += r["partial"]
        at = r["attnT"]
        for hli in range(HL):
            attn[b, HL * g + hli] = at[hli].T
    return out, attn
